# revision 4
# baseline (speedup 1.0000x reference)
"""Trainium2 Bass kernel for nn_L2GESRModule.

Reference computation:
    Fh_conv = Fh @ Wh + bh            (dead: only used via ones_like)
    ESF     = ones_like(Fh_conv)      -> gather indices are a fixed shift
    Y       = Fl @ Wl + bl
    out[b,i,j,:] = Y[b, min(i+1,H-1), min(j+1,W-1), :]

One 1x1-conv GEMM on Fl plus a static (+1,+1) clamped-shift, data-parallel
over batch (1 batch element per core). The Fh/Wh/bh branch is never loaded.

The 2e-2 rel-err budget allows fp16 end-to-end: the host casts Fl/Wl to
fp16 and upcasts the fp16 output, halving HBM traffic (~16.8MB/core ->
~47us at the ~358 GB/s per-core limit). The host also pre-transposes Fl to
[CIN, P] (padded by one zero column) so the kernel needs no PE transposes:
contiguous X^T column blocks are the stationary matmul operand directly
(contiguous -> FWL stays enabled, LDWEIGHTS hides under the matmul).

Flat-pixel layout: image = 16384 pixels; out[O] = Y[O + 129] except col-127
cells (O%128==127) which need Y[O + 128] (clamped col), and the last row
which duplicates row H-2.

Chunks of CH=128*GK pixels, window W0 = O0+129. Group g multiplies
xt[:, kc, g] = FlT[kc*128:+128, W0+g*128 : +128] so psum partition j holds
pixel W0 + g*128 + j and ybig[j, g] = Y[O0 + 129 + g*128 + j]. Stores use
dst "(k p) c -> p k c": partition j, slot g -> out[O0 + g*128 + j], i.e.
ybig[j, g] lands at out pixel O0 + g*128 + j + ... matching the +129 shift.
Col-127 out pixels are exactly partition j=127 (O0, g*128 both = 0 mod 128)
and need Y[O+128] = ybig[126, g], so the store writes partition 127 of the
destination from SBUF partition 126 -- no masked patch ops needed at all.
The last chunk has only 15 valid groups (its window would run past the
input; one padded garbage column is still read at g=14, j=127 but that
output position is patched by the 126-row store). The final output row
duplicates row H-2 = slot g=14 of the last chunk.

PSUM evacuation (fp32 psum -> fp16 SBUF, ~4.2M elem/core) is round-robined
across DVE, ACT and Pool so no single engine bottlenecks. bl is zero for
this module (spec fill=zeros), so the default build evacuates with plain
copies; a with_bias build variant (tensor_add on DVE/Pool) is selected at
runtime if bl is ever nonzero.

Loads go out on the SP HWDGE ring (nc.sync), stores on the ACT HWDGE ring
(nc.scalar) so both physical descriptor rings / all 16 SDMA engines run.
"""

import numpy as np

import concourse.bacc as bacc
import concourse.mybir as mybir
from concourse import bass_utils, tile

B, H, W, CIN, COUT = 8, 128, 128, 256, 256
N_CORES = 8
GK = 16                    # pixel groups (of 128) per chunk
PW = H * W + 1             # padded FlT width


def build_nc(with_bias: bool = False, n_rows: int = H):
    f16 = mybir.dt.float16
    f32 = mybir.dt.float32
    P = n_rows * W  # total pixels per image
    CH = 128 * GK   # pixels per chunk
    assert P % CH == 0 and P >= 2 * CH
    n_chunks = P // CH

    nc = bacc.Bacc("TRN2", target_bir_lowering=False, debug=False)
    FlT = nc.dram_tensor("FlT", [CIN, P + 1], f16, kind="ExternalInput").ap()
    Wl = nc.dram_tensor("Wl", [CIN, COUT], f16, kind="ExternalInput").ap()
    if with_bias:
        blb = nc.dram_tensor("blb", [128, COUT], f32, kind="ExternalInput").ap()
    out = nc.dram_tensor("out", [P, COUT], f16, kind="ExternalOutput").ap()

    with tile.TileContext(nc) as tc:
        with (
            tc.tile_pool(name="consts", bufs=1) as consts,
            tc.tile_pool(name="xt", bufs=4) as xt_pool,
            tc.tile_pool(name="yout", bufs=4) as yout_pool,
            tc.tile_pool(name="py", bufs=8, space="PSUM") as py_pool,
        ):
            # Wl as two K-chunks: w_sb[c, kc, n] = Wl[kc*128 + c, n]
            w_sb = consts.tile([128, 2, COUT], f16)
            nc.sync.dma_start(w_sb, Wl.rearrange("(kc kp) n -> kp kc n", kp=128))
            if with_bias:
                bias_sb = consts.tile([128, COUT], f32)
                nc.sync.dma_start(bias_sb, blb)

            def evac(g, dst, py):
                """dst = py (+ bias), alternating DVE/ACT so neither
                bottlenecks (Pool cannot read PSUM)."""
                if with_bias:
                    nc.vector.tensor_add(dst, py, bias_sb)
                elif g % 2 == 0:
                    nc.vector.tensor_copy(dst, py)
                else:
                    nc.scalar.copy(dst, py)

            h = GK // 2
            for c in range(n_chunks):
                O0 = CH * c
                W0 = O0 + 129
                last = c == n_chunks - 1
                ng = GK - 1 if last else GK  # last chunk: 15 valid groups
                nw = CH - 128 if last else CH
                xt = xt_pool.tile([128, 2, ng, 128], f16, tag="xt")
                src_w = FlT[:, W0 : W0 + nw].rearrange(
                    "(kc p) (g j) -> p kc g j", p=128, j=128
                )
                nc.sync.dma_start(xt[:, 0], src_w[:, 0])
                nc.sync.dma_start(xt[:, 1], src_w[:, 1])
                ybig = yout_pool.tile([128, ng, COUT], f16, tag="yout")
                dst_w = out[O0 : O0 + nw].rearrange("(k p) c -> p k c", p=128)
                for g in range(ng):
                    py = py_pool.tile([128, COUT], f32, tag="py")
                    nc.tensor.matmul(py, xt[:, 0, g], w_sb[:, 0], start=True, stop=False)
                    nc.tensor.matmul(py, xt[:, 1, g], w_sb[:, 1], start=False, stop=True)
                    evac(g, ybig[:, g], py)
                    if g == h - 1:
                        nc.scalar.dma_start(dst_w[0:127, 0:h], ybig[0:127, 0:h])
                nc.scalar.dma_start(dst_w[0:127, h:ng], ybig[0:127, h:ng])
                # col-127 out pixels (partition 127) take the clamped value
                # Y[O+128], which lives on SBUF partition 126
                nc.scalar.dma_start(dst_w[127:128, 0:ng], ybig[126:127, 0:ng])
                if last:
                    # final row duplicates row H-2 = slot g=14 (incl. its patch)
                    dst_f = out[P - 128 : P].rearrange("(k p) c -> p k c", p=128)
                    nc.scalar.dma_start(dst_f[0:127, 0], ybig[0:127, ng - 1])
                    nc.scalar.dma_start(dst_f[127:128, 0], ybig[126:127, ng - 1])

    nc.compile()
    return nc


_cache: dict = {}


def _get_nc(with_bias: bool = False):
    key = ("bias", with_bias)
    if key not in _cache:
        _cache[key] = build_nc(with_bias)
    return _cache[key]


def make_in_maps(Fl, Wl, bl):
    Fl = np.asarray(Fl, dtype=np.float32)
    bl = np.asarray(bl, dtype=np.float32)
    with_bias = bool(np.any(bl))
    Wl16 = np.ascontiguousarray(np.asarray(Wl).astype(np.float16))
    P = H * W
    in_maps = []
    for b in range(B):
        FlT = np.zeros((CIN, P + 1), dtype=np.float16)
        FlT[:, :P] = Fl[b].reshape(P, CIN).T
        m = {"FlT": FlT, "Wl": Wl16}
        if with_bias:
            m["blb"] = np.ascontiguousarray(
                np.broadcast_to(bl, (128, COUT)).astype(np.float32)
            )
        in_maps.append(m)
    return with_bias, in_maps


def kernel(Fh, Fl, Wh, bh, Wl, bl):
    with_bias, in_maps = make_in_maps(Fl, Wl, bl)
    nc = _get_nc(with_bias)
    res = bass_utils.run_bass_kernel_spmd(nc, in_maps, core_ids=list(range(N_CORES)))
    return np.stack(
        [
            res.results[b]["out"].astype(np.float32).reshape(H, W, COUT)
            for b in range(B)
        ],
        axis=0,
    )


# revision 5
# speedup vs baseline: 5.3209x; 5.3209x over previous
"""Trainium2 Bass kernel for nn_L2GESRModule.

Reference computation:
    Fh_conv = Fh @ Wh + bh            (dead: only used via ones_like)
    ESF     = ones_like(Fh_conv)      -> gather indices are a fixed shift
    Y       = Fl @ Wl + bl
    out[b,i,j,:] = Y[b, min(i+1,H-1), min(j+1,W-1), :]

One 1x1-conv GEMM on Fl plus a static (+1,+1) clamped-shift, data-parallel
over batch (1 batch element per core). The Fh/Wh/bh branch is never loaded.

The 2e-2 rel-err budget allows fp16 end-to-end: the host casts Fl/Wl to
fp16 and upcasts the fp16 output, halving HBM traffic (~16.8MB/core ->
~47us at the ~358 GB/s per-core limit). The host also pre-transposes Fl to
FlT [CIN, P+129] (zero-padded) so the kernel needs no PE transposes: X^T
column slices are the stationary matmul operand directly.

Flat-pixel layout: image = 16384 pixels; out[O] = Y[O + 129] except col-127
cells (O%128==127) which need Y[O + 128] (clamped col), and the last row
which duplicates row H-2.

Chunks of CH=128*GK pixels, window W0 = O0+129 (the zero padding keeps the
last chunk's window in bounds, so all chunks are uniform). Group g's
stationary operand is xt[:, kc, :, g] (column j stride GK); psum partition
j then holds pixel W0 + j*GK + g, i.e. ybig[j, g] = Y[O0 + 129 + j*GK + g]
-- GK *consecutive* out pixels per partition -> GK*0.5 KB contiguous per
partition on the store (8 KB descriptors; interleaved layouts with 512 B
descriptors measured ~25x slower on the HWDGE store path). Col-127 out
pixels are the last slot on every (128//GK)'th partition and take the
previous slot's value via a masked copy_predicated. The last chunk's
partitions 120-127 compute on padding garbage and are simply not stored;
the final output row duplicates row H-2 (partitions 112-119).

PSUM evacuation (fp32 psum -> fp16 SBUF, ~4.2M elem/core) alternates
between DVE and ACT so neither bottlenecks (Pool cannot read PSUM). bl is
zero for this module (spec fill=zeros), so the default build evacuates
with plain copies; a with_bias build variant (tensor_add on DVE) is
selected at runtime if bl is ever nonzero.

Loads go out on the SP HWDGE ring (nc.sync), stores on the ACT HWDGE ring
(nc.scalar) so both physical descriptor rings / all 16 SDMA engines run.
"""

import numpy as np

import concourse.bacc as bacc
import concourse.mybir as mybir
from concourse import bass_utils, tile

B, H, W, CIN, COUT = 8, 128, 128, 256, 256
N_CORES = 8
GK = 16                    # pixel-slots per partition per chunk
PW = H * W + 129           # padded FlT width


def build_nc(with_bias: bool = False, n_rows: int = H):
    f16 = mybir.dt.float16
    f32 = mybir.dt.float32
    P = n_rows * W  # total pixels per image
    CH = 128 * GK   # pixels per chunk
    assert P % CH == 0 and P >= 2 * CH
    assert 128 % GK == 0
    n_chunks = P // CH

    nc = bacc.Bacc("TRN2", target_bir_lowering=False, debug=False)
    FlT = nc.dram_tensor("FlT", [CIN, P + 129], f16, kind="ExternalInput").ap()
    Wl = nc.dram_tensor("Wl", [CIN, COUT], f16, kind="ExternalInput").ap()
    if with_bias:
        blb = nc.dram_tensor("blb", [128, COUT], f32, kind="ExternalInput").ap()
    # mask over partitions whose last slot holds a col-127 pixel: engines
    # cannot address strided partitions, so the patch is a predicated copy
    msk = nc.dram_tensor("msk", [128, COUT], mybir.dt.uint8, kind="ExternalInput").ap()
    out = nc.dram_tensor("out", [P, COUT], f16, kind="ExternalOutput").ap()

    with tile.TileContext(nc) as tc:
        with (
            tc.tile_pool(name="consts", bufs=1) as consts,
            tc.tile_pool(name="xt", bufs=4) as xt_pool,
            tc.tile_pool(name="yout", bufs=4) as yout_pool,
            tc.tile_pool(name="py", bufs=8, space="PSUM") as py_pool,
        ):
            # Wl as two K-chunks: w_sb[c, kc, n] = Wl[kc*128 + c, n]
            w_sb = consts.tile([128, 2, COUT], f16)
            nc.sync.dma_start(w_sb, Wl.rearrange("(kc kp) n -> kp kc n", kp=128))
            if with_bias:
                bias_sb = consts.tile([128, COUT], f32)
                nc.sync.dma_start(bias_sb, blb)
            msk_sb = consts.tile([128, COUT], mybir.dt.uint8)
            nc.sync.dma_start(msk_sb, msk)

            def evac(g, dst, py):
                """dst = py (+ bias), alternating DVE/ACT so neither
                bottlenecks (Pool cannot read PSUM)."""
                if with_bias:
                    nc.vector.tensor_add(dst, py, bias_sb)
                elif g % 2 == 0:
                    nc.vector.tensor_copy(dst, py)
                else:
                    nc.scalar.copy(dst, py)

            h = GK // 2
            tail0 = max(h, GK - 4)
            for c in range(n_chunks):
                O0 = CH * c
                W0 = O0 + 129
                # last chunk: partitions 120+ compute on padding, not stored
                nps = (CH - 128) // GK if c == n_chunks - 1 else 128
                xt = xt_pool.tile([128, 2, 128, GK], f16, tag="xt")
                src_w = FlT[:, W0 : W0 + CH].rearrange(
                    "(kc p) (j g) -> p kc j g", p=128, g=GK
                )
                nc.sync.dma_start(xt[:, 0], src_w[:, 0])
                nc.sync.dma_start(xt[:, 1], src_w[:, 1])
                ybig = yout_pool.tile([128, GK, COUT], f16, tag="yout")
                dst_w = out[O0 : O0 + nps * GK].rearrange("(p k) c -> p k c", k=GK)
                for g in range(GK):
                    py = py_pool.tile([128, COUT], f32, tag="py")
                    nc.tensor.matmul(py, xt[:, 0, :, g], w_sb[:, 0], start=True, stop=False)
                    nc.tensor.matmul(py, xt[:, 1, :, g], w_sb[:, 1], start=False, stop=True)
                    evac(g, ybig[:, g], py)
                    if g == h - 1:
                        nc.scalar.dma_start(dst_w[0:nps, 0:h], ybig[0:nps, 0:h])
                    if GK - 4 > h and g == GK - 5:
                        nc.scalar.dma_start(
                            dst_w[0:nps, h : GK - 4], ybig[0:nps, h : GK - 4]
                        )
                # col-127 cells (last slot on masked partitions) duplicate the
                # col-126 value (previous slot): masked predicated copy
                nc.vector.copy_predicated(ybig[:, GK - 1], msk_sb, ybig[:, GK - 2])
                nc.scalar.dma_start(dst_w[0:nps, tail0:GK], ybig[0:nps, tail0:GK])
                if c == n_chunks - 1:
                    # final row duplicates row H-2 = out [P-256, P-128), which
                    # lives on partitions 112..119 (after the col-127 patch)
                    nrp = 128 // GK
                    nc.scalar.dma_start(
                        out[P - 128 : P].rearrange("(p k) c -> p k c", k=GK),
                        ybig[nps - nrp : nps],
                    )

    nc.compile()
    return nc


_cache: dict = {}


def _get_nc(with_bias: bool = False):
    key = ("bias", with_bias)
    if key not in _cache:
        _cache[key] = build_nc(with_bias)
    return _cache[key]


def make_mask():
    # partition j's last slot holds pixel GK*j + GK-1; it is a col-127 pixel
    # iff (GK*j + GK-1) % 128 == 127, i.e. j % (128//GK) == 128//GK - 1
    m = np.zeros((128, COUT), dtype=np.uint8)
    step = 128 // GK
    m[step - 1 :: step, :] = 1
    return m


def make_in_maps(Fl, Wl, bl):
    Fl = np.asarray(Fl, dtype=np.float32)
    bl = np.asarray(bl, dtype=np.float32)
    with_bias = bool(np.any(bl))
    Wl16 = np.ascontiguousarray(np.asarray(Wl).astype(np.float16))
    msk_np = make_mask()
    P = H * W
    in_maps = []
    for b in range(B):
        FlT = np.zeros((CIN, PW), dtype=np.float16)
        FlT[:, :P] = Fl[b].reshape(P, CIN).T
        m = {"FlT": FlT, "Wl": Wl16, "msk": msk_np}
        if with_bias:
            m["blb"] = np.ascontiguousarray(
                np.broadcast_to(bl, (128, COUT)).astype(np.float32)
            )
        in_maps.append(m)
    return with_bias, in_maps


def kernel(Fh, Fl, Wh, bh, Wl, bl):
    with_bias, in_maps = make_in_maps(Fl, Wl, bl)
    nc = _get_nc(with_bias)
    res = bass_utils.run_bass_kernel_spmd(nc, in_maps, core_ids=list(range(N_CORES)))
    return np.stack(
        [
            res.results[b]["out"].astype(np.float32).reshape(H, W, COUT)
            for b in range(B)
        ],
        axis=0,
    )


# revision 6
# speedup vs baseline: 5.6309x; 1.0583x over previous
"""Trainium2 Bass kernel for nn_L2GESRModule.

Reference computation:
    Fh_conv = Fh @ Wh + bh            (dead: only used via ones_like)
    ESF     = ones_like(Fh_conv)      -> gather indices are a fixed shift
    Y       = Fl @ Wl + bl
    out[b,i,j,:] = Y[b, min(i+1,H-1), min(j+1,W-1), :]

One 1x1-conv GEMM on Fl plus a static (+1,+1) clamped-shift, data-parallel
over batch (1 batch element per core). The Fh/Wh/bh branch is never loaded.

The 2e-2 rel-err budget allows fp16 end-to-end: the host casts Fl/Wl to
fp16 and upcasts the fp16 output, halving HBM traffic (~16.8MB/core ->
~47us at the ~358 GB/s per-core limit). The host also pre-transposes Fl to
FlT [CIN, P+129] (zero-padded) so the kernel needs no PE transposes: X^T
column slices are the stationary matmul operand directly.

Flat-pixel layout: image = 16384 pixels; out[O] = Y[O + 129] except col-127
cells (O%128==127) which need Y[O + 128] (clamped col), and the last row
which duplicates row H-2.

Chunks of CH=128*GK pixels, window W0 = O0+129 (the zero padding keeps the
last chunk's window in bounds, so all chunks are uniform). Group g's
stationary operand is xt[:, kc, :, g] (column j stride GK); psum partition
j then holds pixel W0 + j*GK + g, i.e. ybig[j, g] = Y[O0 + 129 + j*GK + g]
-- GK *consecutive* out pixels per partition -> GK*0.5 KB contiguous per
partition on the store (8 KB descriptors; interleaved layouts with 512 B
descriptors measured ~25x slower on the HWDGE store path). Col-127 out
pixels are the last slot on every (128//GK)'th partition and take the
previous slot's value via a masked copy_predicated. The last chunk's
partitions 120-127 compute on padding garbage and are simply not stored;
the final output row duplicates row H-2 (partitions 112-119).

PSUM evacuation (fp32 psum -> fp16 SBUF, ~4.2M elem/core) alternates
between DVE and ACT so neither bottlenecks (Pool cannot read PSUM). bl is
zero for this module (spec fill=zeros), so the default build evacuates
with plain copies; a with_bias build variant (tensor_add on DVE) is
selected at runtime if bl is ever nonzero.

Loads go out on the SP HWDGE ring (nc.sync), stores on the ACT HWDGE ring
(nc.scalar) so both physical descriptor rings / all 16 SDMA engines run.
"""

import numpy as np

import concourse.bacc as bacc
import concourse.mybir as mybir
from concourse import bass_utils, tile

B, H, W, CIN, COUT = 8, 128, 128, 256, 256
N_CORES = 8
GK = 16                    # pixel-slots per partition per chunk
PW = H * W + 129           # padded FlT width


def build_nc(with_bias: bool = False, n_rows: int = H):
    f16 = mybir.dt.float16
    f32 = mybir.dt.float32
    P = n_rows * W  # total pixels per image
    CH = 128 * GK   # pixels per chunk
    assert P % CH == 0 and P >= 2 * CH
    assert 128 % GK == 0
    n_chunks = P // CH

    nc = bacc.Bacc("TRN2", target_bir_lowering=False, debug=False)
    FlT = nc.dram_tensor("FlT", [CIN, P + 129], f16, kind="ExternalInput").ap()
    Wl = nc.dram_tensor("Wl", [CIN, COUT], f16, kind="ExternalInput").ap()
    if with_bias:
        blb = nc.dram_tensor("blb", [128, COUT], f32, kind="ExternalInput").ap()
    # mask over partitions whose last slot holds a col-127 pixel: engines
    # cannot address strided partitions, so the patch is a predicated copy
    msk = nc.dram_tensor("msk", [128, COUT], mybir.dt.uint8, kind="ExternalInput").ap()
    out = nc.dram_tensor("out", [P, COUT], f16, kind="ExternalOutput").ap()

    with tile.TileContext(nc) as tc:
        with (
            tc.tile_pool(name="consts", bufs=1) as consts,
            tc.tile_pool(name="xt", bufs=4) as xt_pool,
            tc.tile_pool(name="yout", bufs=4) as yout_pool,
            tc.tile_pool(name="py", bufs=8, space="PSUM") as py_pool,
        ):
            # Wl as two K-chunks: w_sb[c, kc, n] = Wl[kc*128 + c, n]
            w_sb = consts.tile([128, 2, COUT], f16)
            nc.sync.dma_start(w_sb, Wl.rearrange("(kc kp) n -> kp kc n", kp=128))
            if with_bias:
                bias_sb = consts.tile([128, COUT], f32)
                nc.sync.dma_start(bias_sb, blb)
            msk_sb = consts.tile([128, COUT], mybir.dt.uint8)
            nc.sync.dma_start(msk_sb, msk)

            def evac(g, dst, py):
                """dst = py (+ bias), split DVE/ACT ~9:7 (their measured
                PSUM-read rates match) so neither bottlenecks (Pool cannot
                read PSUM)."""
                if with_bias:
                    nc.vector.tensor_add(dst, py, bias_sb)
                elif g % 16 < 9:
                    nc.vector.tensor_copy(dst, py)
                else:
                    nc.scalar.copy(dst, py)

            for c in range(n_chunks):
                O0 = CH * c
                W0 = O0 + 129
                # last chunk: partitions 120+ compute on padding, not stored
                nps = (CH - 128) // GK if c == n_chunks - 1 else 128
                xt = xt_pool.tile([128, 2, 128, GK], f16, tag="xt")
                src_w = FlT[:, W0 : W0 + CH].rearrange(
                    "(kc p) (j g) -> p kc j g", p=128, g=GK
                )
                nc.sync.dma_start(xt, src_w)
                ybig = yout_pool.tile([128, GK, COUT], f16, tag="yout")
                dst_w = out[O0 : O0 + nps * GK].rearrange("(p k) c -> p k c", k=GK)
                for g in range(GK):
                    py = py_pool.tile([128, COUT], f32, tag="py")
                    nc.tensor.matmul(py, xt[:, 0, :, g], w_sb[:, 0], start=True, stop=False)
                    nc.tensor.matmul(py, xt[:, 1, :, g], w_sb[:, 1], start=False, stop=True)
                    evac(g, ybig[:, g], py)
                # col-127 cells (last slot on masked partitions) duplicate the
                # col-126 value (previous slot): masked predicated copy
                nc.vector.copy_predicated(ybig[:, GK - 1], msk_sb, ybig[:, GK - 2])
                nc.scalar.dma_start(dst_w[0:nps], ybig[0:nps])
                if c == n_chunks - 1:
                    # final row duplicates row H-2 = out [P-256, P-128), which
                    # lives on partitions 112..119 (after the col-127 patch)
                    nrp = 128 // GK
                    nc.scalar.dma_start(
                        out[P - 128 : P].rearrange("(p k) c -> p k c", k=GK),
                        ybig[nps - nrp : nps],
                    )

    nc.compile()
    return nc


_cache: dict = {}


def _get_nc(with_bias: bool = False):
    key = ("bias", with_bias)
    if key not in _cache:
        _cache[key] = build_nc(with_bias)
    return _cache[key]


def make_mask():
    # partition j's last slot holds pixel GK*j + GK-1; it is a col-127 pixel
    # iff (GK*j + GK-1) % 128 == 127, i.e. j % (128//GK) == 128//GK - 1
    m = np.zeros((128, COUT), dtype=np.uint8)
    step = 128 // GK
    m[step - 1 :: step, :] = 1
    return m


def make_in_maps(Fl, Wl, bl):
    Fl = np.asarray(Fl, dtype=np.float32)
    bl = np.asarray(bl, dtype=np.float32)
    with_bias = bool(np.any(bl))
    Wl16 = np.ascontiguousarray(np.asarray(Wl).astype(np.float16))
    msk_np = make_mask()
    P = H * W
    in_maps = []
    for b in range(B):
        FlT = np.zeros((CIN, PW), dtype=np.float16)
        FlT[:, :P] = Fl[b].reshape(P, CIN).T
        m = {"FlT": FlT, "Wl": Wl16, "msk": msk_np}
        if with_bias:
            m["blb"] = np.ascontiguousarray(
                np.broadcast_to(bl, (128, COUT)).astype(np.float32)
            )
        in_maps.append(m)
    return with_bias, in_maps


def kernel(Fh, Fl, Wh, bh, Wl, bl):
    with_bias, in_maps = make_in_maps(Fl, Wl, bl)
    nc = _get_nc(with_bias)
    res = bass_utils.run_bass_kernel_spmd(nc, in_maps, core_ids=list(range(N_CORES)))
    return np.stack(
        [
            res.results[b]["out"].astype(np.float32).reshape(H, W, COUT)
            for b in range(B)
        ],
        axis=0,
    )


# revision 9
# speedup vs baseline: 5.8990x; 1.0476x over previous
"""Trainium2 Bass kernel for nn_L2GESRModule.

Reference computation:
    Fh_conv = Fh @ Wh + bh            (dead: only used via ones_like)
    ESF     = ones_like(Fh_conv)      -> gather indices are a fixed shift
    Y       = Fl @ Wl + bl
    out[b,i,j,:] = Y[b, min(i+1,H-1), min(j+1,W-1), :]

One 1x1-conv GEMM on Fl plus a static (+1,+1) clamped-shift, data-parallel
over batch (1 batch element per core). The Fh/Wh/bh branch is never loaded.

The 2e-2 rel-err budget allows fp16 end-to-end: the host casts Fl/Wl to
fp16 and upcasts the fp16 output, halving HBM traffic (~16.8MB/core ->
~47us at the ~358 GB/s per-core limit). The host also pre-transposes Fl to
FlT [CIN, P+129] (zero-padded) so the kernel needs no PE transposes: X^T
column slices are the stationary matmul operand directly.

Flat-pixel layout: image = 16384 pixels; out[O] = Y[O + 129] except col-127
cells (O%128==127) which need Y[O + 128] (clamped col), and the last row
which duplicates row H-2.

Chunks of CH=128*GK pixels, window W0 = O0+129 (the zero padding keeps the
last chunk's window in bounds, so all chunks are uniform). Group g's
stationary operand is xt[:, kc, :, g] (column j stride GK); psum partition
j then holds pixel W0 + j*GK + g, i.e. ybig[j, g] = Y[O0 + 129 + j*GK + g]
-- GK *consecutive* out pixels per partition -> GK*0.5 KB contiguous per
partition on the store (8 KB descriptors; interleaved layouts with 512 B
descriptors measured ~25x slower on the HWDGE store path). Col-127 out
pixels are the last slot on every (128//GK)'th partition and take the
previous slot's value via a masked copy_predicated. The last chunk's
partitions 120-127 compute on padding garbage and are simply not stored;
the final output row duplicates row H-2 (partitions 112-119).

PSUM evacuation (fp32 psum -> fp16 SBUF, ~4.2M elem/core) alternates
between DVE and ACT so neither bottlenecks (Pool cannot read PSUM). bl is
zero for this module (spec fill=zeros), so the default build evacuates
with plain copies; a with_bias build variant (tensor_add on DVE) is
selected at runtime if bl is ever nonzero.

Loads go out on the SP HWDGE ring (nc.sync), stores on the ACT HWDGE ring
(nc.scalar) so both physical descriptor rings / all 16 SDMA engines run.
"""

import numpy as np

import concourse.bacc as bacc
import concourse.mybir as mybir
from concourse import bass_utils, tile

B, H, W, CIN, COUT = 8, 128, 128, 256, 256
N_CORES = 8
GK = 16                    # pixel-slots per partition per chunk
PW = H * W + 129           # padded FlT width


def build_nc(with_bias: bool = False, n_rows: int = H):
    f16 = mybir.dt.float16
    f32 = mybir.dt.float32
    P = n_rows * W  # total pixels per image
    CH = 128 * GK   # pixels per chunk
    assert P % CH == 0 and P >= 2 * CH
    assert 128 % GK == 0
    n_chunks = P // CH

    nc = bacc.Bacc("TRN2", target_bir_lowering=False, debug=False)
    FlT = nc.dram_tensor("FlT", [CIN, P + 129], f16, kind="ExternalInput").ap()
    Wl = nc.dram_tensor("Wl", [CIN, COUT], f16, kind="ExternalInput").ap()
    if with_bias:
        blb = nc.dram_tensor("blb", [128, COUT], f32, kind="ExternalInput").ap()
    # mask over partitions whose last slot holds a col-127 pixel: engines
    # cannot address strided partitions, so the patch is a predicated copy
    msk = nc.dram_tensor("msk", [128, COUT], mybir.dt.uint8, kind="ExternalInput").ap()
    out = nc.dram_tensor("out", [P, COUT], f16, kind="ExternalOutput").ap()

    with tile.TileContext(nc) as tc:
        with (
            tc.tile_pool(name="consts", bufs=1) as consts,
            tc.tile_pool(name="xt", bufs=6) as xt_pool,
            tc.tile_pool(name="yout", bufs=5) as yout_pool,
            tc.tile_pool(name="py", bufs=4, space="PSUM") as py_pool,
        ):
            # Wl as two K-chunks: w_sb[c, kc, n] = Wl[kc*128 + c, n]
            w_sb = consts.tile([128, 2, COUT], f16)
            nc.sync.dma_start(w_sb, Wl.rearrange("(kc kp) n -> kp kc n", kp=128))
            if with_bias:
                bias_sb = consts.tile([128, COUT], f32)
                nc.sync.dma_start(bias_sb, blb)
            msk_sb = consts.tile([128, COUT], mybir.dt.uint8)
            nc.sync.dma_start(msk_sb, msk)

            def evac(q, dst, py):
                """dst = py (+ bias) for GE groups at once (amortizes the
                ~120/172-cycle per-op overhead), alternating DVE/ACT so
                neither bottlenecks (Pool cannot read PSUM)."""
                if with_bias:
                    nc.vector.tensor_add(dst, py, bias_sb)
                elif q % 2 == 0:
                    nc.vector.tensor_copy(dst, py)
                else:
                    nc.scalar.copy(dst, py)

            GE = 4  # groups per PSUM tile / evac instruction (2 banks)
            for c in range(n_chunks):
                O0 = CH * c
                W0 = O0 + 129
                # last chunk: partitions 120+ compute on padding, not stored
                nps = (CH - 128) // GK if c == n_chunks - 1 else 128
                xt = xt_pool.tile([128, 2, 128, GK], f16, tag="xt")
                src_w = FlT[:, W0 : W0 + CH].rearrange(
                    "(kc p) (j g) -> p kc j g", p=128, g=GK
                )
                if c == 0:
                    # split the first load so compute ramps earlier
                    nc.sync.dma_start(xt[:, 0], src_w[:, 0])
                    nc.sync.dma_start(xt[:, 1], src_w[:, 1])
                else:
                    nc.sync.dma_start(xt, src_w)
                ybig = yout_pool.tile([128, GK, COUT], f16, tag="yout")
                dst_w = out[O0 : O0 + nps * GK].rearrange("(p k) c -> p k c", k=GK)
                for q in range(GK // GE):
                    py = py_pool.tile([128, GE, COUT], f32, tag="py")
                    for gg in range(GE):
                        g = q * GE + gg
                        nc.tensor.matmul(py[:, gg], xt[:, 0, :, g], w_sb[:, 0], start=True, stop=False)
                        nc.tensor.matmul(py[:, gg], xt[:, 1, :, g], w_sb[:, 1], start=False, stop=True)
                    evac(q, ybig[:, q * GE : (q + 1) * GE], py)
                # col-127 cells (last slot on masked partitions) duplicate the
                # col-126 value (previous slot): masked predicated copy
                nc.vector.copy_predicated(ybig[:, GK - 1], msk_sb, ybig[:, GK - 2])
                nc.scalar.dma_start(dst_w[0:nps], ybig[0:nps])
                if c == n_chunks - 1:
                    # final row duplicates row H-2 = out [P-256, P-128), which
                    # lives on partitions 112..119 (after the col-127 patch)
                    nrp = 128 // GK
                    nc.scalar.dma_start(
                        out[P - 128 : P].rearrange("(p k) c -> p k c", k=GK),
                        ybig[nps - nrp : nps],
                    )

    nc.compile()
    return nc


_cache: dict = {}


def _get_nc(with_bias: bool = False):
    key = ("bias", with_bias)
    if key not in _cache:
        _cache[key] = build_nc(with_bias)
    return _cache[key]


def make_mask():
    # partition j's last slot holds pixel GK*j + GK-1; it is a col-127 pixel
    # iff (GK*j + GK-1) % 128 == 127, i.e. j % (128//GK) == 128//GK - 1
    m = np.zeros((128, COUT), dtype=np.uint8)
    step = 128 // GK
    m[step - 1 :: step, :] = 1
    return m


def make_in_maps(Fl, Wl, bl):
    Fl = np.asarray(Fl, dtype=np.float32)
    bl = np.asarray(bl, dtype=np.float32)
    with_bias = bool(np.any(bl))
    Wl16 = np.ascontiguousarray(np.asarray(Wl).astype(np.float16))
    msk_np = make_mask()
    P = H * W
    in_maps = []
    for b in range(B):
        FlT = np.zeros((CIN, PW), dtype=np.float16)
        FlT[:, :P] = Fl[b].reshape(P, CIN).T
        m = {"FlT": FlT, "Wl": Wl16, "msk": msk_np}
        if with_bias:
            m["blb"] = np.ascontiguousarray(
                np.broadcast_to(bl, (128, COUT)).astype(np.float32)
            )
        in_maps.append(m)
    return with_bias, in_maps


def kernel(Fh, Fl, Wh, bh, Wl, bl):
    with_bias, in_maps = make_in_maps(Fl, Wl, bl)
    nc = _get_nc(with_bias)
    res = bass_utils.run_bass_kernel_spmd(nc, in_maps, core_ids=list(range(N_CORES)))
    return np.stack(
        [
            res.results[b]["out"].astype(np.float32).reshape(H, W, COUT)
            for b in range(B)
        ],
        axis=0,
    )
